# revision 5
# baseline (speedup 1.0000x reference)
"""Trainium2 Bass kernel for nn_Attention_24704651887034.

Dense ViT-style attention block (B=16, N=1024, C=768, H=12 heads, 2D RoPE),
data-parallel over batch across 8 NeuronCores (2 batch items per core, no
collectives).

v2 — HAM-warm dense-PE schedule:
  * The PE array clock-gates to 1.2 GHz unless continuously busy (~3.4us
    windows).  The kernel therefore emits a single dense PE instruction
    stream: attention steps are interleaved with "filler" matmul closures
    (next item's QKV/V, previous item's proj) so the PE never idles while
    the Scalar engine computes softmax exps.
  * Score matmuls for the two heads of a pair are emitted adjacently on
    row-tiles (0,0)/(64,0) so the 64-contraction matmuls run concurrently
    in the two halves of the PE array.
  * exp is fused over kc-pairs ([128,1024] ACT ops) to halve ACT overhead.
  * Softmax denominators stay in PSUM row 64 ([v|1] ones trick); per
    (pair, q-half) they are inverted with the single-pass DVE
    reciprocal_approx_fast, DMA-broadcast, and fused into the PSUM->SBUF
    evacuation multiply (no separate normalize pass, nothing on the
    critical path at the proj boundary).
"""

from collections import deque

import numpy as np

_B, _N, _C, _H = 16, 1024, 768, 12
_HD, _DR = 64, 32
_ROPE_BASE = 10000.0
_NCORES = 8
_BL = _B // _NCORES  # batch items per core

_NC6 = _C // 128      # 6 contraction chunks

_nc_cache = {}


def _split_excess_waits(nc, max_waits=1):
    """Walrus in this toolchain accepts at most one sync-wait command per
    instruction; Tile's tail drain (and occasionally the scheduler) emits
    more. Split the excess onto same-engine NOPs inserted just before."""
    from concourse import mybir

    for f in nc.m.functions:
        for blk in f.blocks:
            insts = blk.instructions
            i = 0
            while i < len(insts):
                ins = insts[i]
                si = getattr(ins, "sync_info", None)
                if si is not None and len(si.on_wait) > max_waits:
                    excess = si.on_wait[max_waits:]
                    ins.sync_info = mybir.SyncInfo(
                        on_wait=list(si.on_wait[:max_waits]),
                        on_update=list(si.on_update),
                    )
                    for j, w in enumerate(excess):
                        nop = mybir.InstNoOp(
                            name=f"{ins.name}-sw{j}", engine=ins.engine
                        )
                        nop.sync_info = mybir.SyncInfo(on_wait=[w], on_update=[])
                        insts.insert(i, nop)
                        i += 1
                i += 1
    return nc


def _build(has_bias):
    from contextlib import ExitStack

    import concourse.bass as bass
    import concourse.tile as tile
    from concourse import mybir

    BF = mybir.dt.bfloat16
    F32 = mybir.dt.float32
    Exp = mybir.ActivationFunctionType.Exp
    N, C, H = _N, _C, _H
    BL = _BL

    nc = bass.Bass("TRN2", target_bir_lowering=False, debug=False)
    x_d = nc.dram_tensor("x", [BL * N, C], BF, kind="ExternalInput").ap()
    wq_d = nc.dram_tensor("wq", [C, 3 * C], BF, kind="ExternalInput").ap()
    wp_d = nc.dram_tensor("wp", [C, C], BF, kind="ExternalInput").ap()
    cos_d = nc.dram_tensor("cosm", [BL * 128, N], BF, kind="ExternalInput").ap()
    sin_d = nc.dram_tensor("ssinm", [BL * 128, N], BF, kind="ExternalInput").ap()
    if has_bias:
        bqk_d = nc.dram_tensor("bqk", [1, 2 * C], BF, kind="ExternalInput").ap()
        bv_d = nc.dram_tensor("bv", [1, C], BF, kind="ExternalInput").ap()
        bp_d = nc.dram_tensor("bp", [1, C], BF, kind="ExternalInput").ap()
    out_d = nc.dram_tensor("out", [BL * N, C], F32, kind="ExternalOutput").ap()

    SH_MASK = [(i + 16) % 32 for i in range(32)]
    F_ORDER = [0, 6, 1, 7, 2, 8, 3, 9, 4, 10, 5, 11]

    with tile.TileContext(nc) as tc, ExitStack() as ctx:
        const = ctx.enter_context(tc.tile_pool(name="const", bufs=1))
        xT_p = ctx.enter_context(tc.tile_pool(name="xT", bufs=2 * _NC6))
        map_p = ctx.enter_context(tc.tile_pool(name="maps", bufs=2))
        raw_p = ctx.enter_context(tc.tile_pool(name="qraw", bufs=2))
        tmp_p = ctx.enter_context(tc.tile_pool(name="rtmp", bufs=3))
        qkr_p = ctx.enter_context(tc.tile_pool(name="qkr", bufs=24))
        v_p = ctx.enter_context(tc.tile_pool(name="v", bufs=16))
        pT_p = ctx.enter_context(tc.tile_pool(name="pT", bufs=5))
        ao_p = ctx.enter_context(tc.tile_pool(name="ao", bufs=12))
        rec_p = ctx.enter_context(tc.tile_pool(name="rec", bufs=2))
        bch_p = ctx.enter_context(tc.tile_pool(name="bch", bufs=2))
        ost_p = ctx.enter_context(tc.tile_pool(name="ost", bufs=3))
        mm_ps = ctx.enter_context(tc.tile_pool(name="mmps", bufs=2, space="PSUM"))
        sc_ps = ctx.enter_context(tc.tile_pool(name="scps", bufs=2, space="PSUM"))
        o_ps = ctx.enter_context(tc.tile_pool(name="ops", bufs=2, space="PSUM"))

        # ---- resident constants: weights ----
        wq_t = []
        for c in range(_NC6):
            t = const.tile([128, 3 * C], BF, tag=f"wq{c}")
            nc.sync.dma_start(t[:], wq_d[c * 128:(c + 1) * 128, :])
            wq_t.append(t)
        wp_t = []
        for c in range(_NC6):
            t = const.tile([128, C], BF, tag=f"wp{c}")
            nc.sync.dma_start(t[:], wp_d[c * 128:(c + 1) * 128, :])
            wp_t.append(t)
        if has_bias:
            bqk_sb = const.tile([1, 2 * C], BF, tag="bqk")
            nc.sync.dma_start(bqk_sb[:], bqk_d[:])
            bv_sb = const.tile([1, C], BF, tag="bv")
            nc.sync.dma_start(bv_sb[:], bv_d[:])
            bp_sb = const.tile([1, C], BF, tag="bp")
            nc.sync.dma_start(bp_sb[:], bp_d[:])
            ones_r = const.tile([1, 512], BF, tag="ones")
            nc.gpsimd.memset(ones_r[:], 1.0)

        # ---- per-item input DMAs, all issued upfront ----
        xT = {}
        cosm = {}
        ssin = {}
        for b in range(BL):
            for c in range(_NC6):
                t = xT_p.tile([128, N], BF, tag="xT", name=f"xT{b}_{c}")
                nc.sync.dma_start(
                    t[:], x_d[b * N:(b + 1) * N, c * 128:(c + 1) * 128],
                    transpose=True,
                )
                xT[b, c] = t
            m = map_p.tile([128, N], BF, tag="cos", name=f"cos{b}")
            nc.sync.dma_start(m[:], cos_d[b * 128:(b + 1) * 128, :])
            cosm[b] = m
            m = map_p.tile([128, N], BF, tag="sin", name=f"sin{b}")
            nc.sync.dma_start(m[:], sin_d[b * 128:(b + 1) * 128, :])
            ssin[b] = m

        qk_r = {}
        v_sb = {}
        ao = {}

        # ---- emitters ----
        def emit_q_half(b, f, t2, raw):
            ps = mm_ps.tile([128, 512], F32, tag="mm", name=f"q{b}_{f}_{t2}")
            for c in range(_NC6):
                nc.tensor.matmul(
                    ps[:],
                    wq_t[c][:, f * 128:(f + 1) * 128],
                    xT[b, c][:, t2 * 512:(t2 + 1) * 512],
                    start=(c == 0),
                    stop=(c == _NC6 - 1 and not has_bias),
                )
            if has_bias:
                nc.tensor.matmul(
                    ps[:],
                    bqk_sb[:, f * 128:(f + 1) * 128],
                    ones_r[:],
                    start=False,
                    stop=True,
                )
            nc.scalar.copy(raw[:, t2 * 512:(t2 + 1) * 512], ps[:])

        def emit_q_rope(b, f, raw):
            r = tmp_p.tile([128, N], BF, tag="rt", name=f"rr{b}_{f}")
            nc.vector.stream_shuffle(r[:], raw[:], SH_MASK)
            tm = tmp_p.tile([128, N], BF, tag="rt", name=f"rm{b}_{f}")
            nc.vector.tensor_mul(tm[:], r[:], ssin[b][:])
            am = tmp_p.tile([128, N], BF, tag="rt", name=f"ra{b}_{f}")
            nc.gpsimd.tensor_mul(am[:], raw[:], cosm[b][:])
            ro = qkr_p.tile([128, N], BF, tag="qkr", name=f"qkr{b}_{f}")
            nc.gpsimd.tensor_add(ro[:], tm[:], am[:])
            qk_r[b, f] = ro

        def mk_q_closures(b, f):
            st = {}

            def c1():
                st["raw"] = raw_p.tile([128, N], BF, tag="qraw", name=f"qw{b}_{f}")
                emit_q_half(b, f, 0, st["raw"])

            def c2():
                emit_q_half(b, f, 1, st["raw"])
                emit_q_rope(b, f, st["raw"])

            return [c1, c2]

        def mk_v_closures(b, t8):
            st = {}

            def grp(f0, fw, first):
                vt = st["vt"]
                vt3 = vt.rearrange("p (h w) -> p h w", w=65)
                if first:
                    nc.gpsimd.memset(vt3[:, :, 64:65], 1.0)
                ps = mm_ps.tile([128, 512], F32, tag="mm", name=f"v{b}_{t8}_{f0}")
                for c in range(_NC6):
                    nc.tensor.matmul(
                        ps[:, :fw],
                        xT[b, c][:, t8 * 128:(t8 + 1) * 128],
                        wq_t[c][:, 2 * C + f0:2 * C + f0 + fw],
                        start=(c == 0),
                        stop=(c == _NC6 - 1 and not has_bias),
                    )
                if has_bias:
                    nc.tensor.matmul(
                        ps[:, :fw],
                        ones_r[:, t8 * 128 % 512:t8 * 128 % 512 + 128],
                        bv_sb[:, f0:f0 + fw],
                        start=False,
                        stop=True,
                    )
                nh = fw // 64
                nc.vector.tensor_copy(
                    vt3[:, f0 // 64:f0 // 64 + nh, 0:64],
                    ps[:, :fw].rearrange("p (h w) -> p h w", w=64),
                )

            def c1():
                st["vt"] = v_p.tile([128, H * 65], BF, tag="v", name=f"v{b}_{t8}")
                v_sb[b, t8] = st["vt"]
                grp(0, 512, True)

            def c2():
                grp(512, 256, False)

            return [c1, c2]

        def mk_p_closure(b, t8, nf):
            def c1():
                ps = mm_ps.tile([128, 512], F32, tag="mm", name=f"p{b}_{t8}_{nf}")
                for jj in range(_NC6):
                    nc.tensor.matmul(
                        ps[:, :384],
                        ao[b, jj][:, t8 * 128:(t8 + 1) * 128],
                        wp_t[jj][:, nf * 384:(nf + 1) * 384],
                        start=(jj == 0),
                        stop=(jj == _NC6 - 1 and not has_bias),
                    )
                if has_bias:
                    nc.tensor.matmul(
                        ps[:, :384],
                        ones_r[:, 0:128],
                        bp_sb[:, nf * 384:(nf + 1) * 384],
                        start=False,
                        stop=True,
                    )
                ot = ost_p.tile([128, 384], F32, tag="ost", name=f"ot{b}_{t8}_{nf}")
                nc.vector.tensor_copy(ot[:], ps[:, :384])
                nc.sync.dma_start(
                    out_d[b * N + t8 * 128:b * N + (t8 + 1) * 128,
                          nf * 384:(nf + 1) * 384],
                    ot[:],
                )

            return [c1]

        # ---- filler machinery ----
        fillers = deque()
        fstate = {"acc": 0.0, "ratio": 0.0}

        def set_fill_ratio(n_points):
            fstate["ratio"] = (len(fillers) / n_points) if n_points else 0.0
            fstate["acc"] = 0.0

        def fill():
            fstate["acc"] += fstate["ratio"]
            while fstate["acc"] >= 1.0 and fillers:
                fillers.popleft()()
                fstate["acc"] -= 1.0

        def flush_fillers():
            while fillers:
                fillers.popleft()()

        # ---- attention pair emitter ----
        def emit_att_pair(b, j):
            qT = qk_r[b, j]
            kT = qk_r[b, 6 + j]
            if (b, j) not in ao:
                ao[b, j] = ao_p.tile([128, N], BF, tag="ao", name=f"ao{b}_{j}")
            for qh in range(2):
                qsl = slice(qh * 512, (qh + 1) * 512)
                o_t = [
                    o_ps.tile([65, 512], F32, tag="o", name=f"o{b}_{j}_{qh}_{h2}")
                    for h2 in range(2)
                ]
                pts = [[None] * 4 for _ in range(2)]
                for i in range(5):
                    if i < 4:
                        scs = [
                            sc_ps.tile([128, 1024], F32, tag="sc",
                                       name=f"s{b}_{j}_{qh}_{i}_{h2}")
                            for h2 in range(2)
                        ]
                        # adjacent row-tiled score pairs: (0,0) then (64,0)
                        for u in range(2):
                            kc = 2 * i + u
                            for h2 in range(2):
                                half = h2 * 64
                                nc.tensor.matmul(
                                    scs[h2][:, u * 512:(u + 1) * 512],
                                    kT[half:half + 64, kc * 128:(kc + 1) * 128],
                                    qT[half:half + 64, qsl],
                                    start=True,
                                    stop=True,
                                )
                        for h2 in range(2):
                            pt = pT_p.tile([128, 1024], BF, tag="pt",
                                           name=f"pt{b}_{j}_{qh}_{i}_{h2}")
                            nc.scalar.activation(pt[:], scs[h2][:], Exp, scale=0.125)
                            pts[h2][i] = pt
                    if i >= 1:
                        for u in range(2):
                            kd = 2 * (i - 1) + u
                            for h2 in range(2):
                                h = 2 * j + h2
                                vt3 = v_sb[b, kd].rearrange("p (h w) -> p h w", w=65)
                                nc.tensor.matmul(
                                    o_t[h2][:],
                                    vt3[:, h, 0:65],
                                    pts[h2][i - 1][:, u * 512:(u + 1) * 512],
                                    start=(kd == 0),
                                    stop=(kd == 7),
                                )
                    fill()
                # denominators + fused normalize/evac
                for h2 in range(2):
                    half = h2 * 64
                    nm = f"{b}_{j}_{qh}_{h2}"
                    dsb = rec_p.tile([1, 512], F32, tag="dr", name=f"dr{nm}")
                    nc.vector.tensor_copy(dsb[:], o_t[h2][64:65, :])
                    d16 = rec_p.tile([16, 32], F32, tag="d16", name=f"d6{nm}")
                    nc.sync.dma_start(
                        d16[:], dsb[:].rearrange("p (a w) -> p a w", a=16)
                    )
                    d16r = rec_p.tile([16, 32], F32, tag="d16r", name=f"d7{nm}")
                    nc.vector.reciprocal(d16r[:], d16[:])
                    rrow = rec_p.tile([1, 512], F32, tag="rrow", name=f"rw{nm}")
                    nc.sync.dma_start(
                        rrow[:].rearrange("p (a w) -> p a w", a=16), d16r[:]
                    )
                    bch = bch_p.tile([64, 512], F32, tag="bch", name=f"bc{nm}")
                    nc.sync.dma_start(
                        bch[:],
                        rrow[:]
                        .rearrange("p (u n) -> p u n", u=1)
                        .broadcast_to((1, 64, 512)),
                    )
                    nc.vector.tensor_mul(
                        ao[b, j][half:half + 64, qsl], o_t[h2][0:64, :], bch[:]
                    )
                fill()

        # ================= main schedule =================
        # item 0: QKV + V inline (PE-dense, ACT idle -> evac on ACT)
        for f in F_ORDER:
            raw = raw_p.tile([128, N], BF, tag="qraw", name=f"qw0_{f}")
            emit_q_half(0, f, 0, raw)
            emit_q_half(0, f, 1, raw)
            emit_q_rope(0, f, raw)
        for t8 in range(8):
            for c in mk_v_closures(0, t8):
                c()

        # attention item 0, filled with item-1 QKV + V
        for f in F_ORDER:
            fillers.extend(mk_q_closures(1, f))
        for t8 in range(8):
            fillers.extend(mk_v_closures(1, t8))
        set_fill_ratio(6 * 2 * 6)
        for j in range(_NC6):
            emit_att_pair(0, j)
        flush_fillers()

        # attention item 1, filled with item-0 proj
        for t8 in range(8):
            for nf in range(2):
                fillers.extend(mk_p_closure(0, t8, nf))
        set_fill_ratio(6 * 2 * 6)
        for j in range(_NC6):
            emit_att_pair(1, j)
        flush_fillers()

        # proj item 1 inline
        for t8 in range(8):
            for nf in range(2):
                for c in mk_p_closure(1, t8, nf):
                    c()
    return _split_excess_waits(nc)


def _get_nc(has_bias):
    if has_bias not in _nc_cache:
        _nc_cache[has_bias] = _build(has_bias)
    return _nc_cache[has_bias]


def _prep_in_maps(x, W_qkv, b_qkv, W_proj, b_proj, pos_h, pos_w):
    import ml_dtypes

    bf16 = ml_dtypes.bfloat16
    has_bias = bool(np.any(b_qkv)) or bool(np.any(b_proj))

    inv = 1.0 / _ROPE_BASE ** (
        np.arange(0, _DR, 2, dtype=np.float32) / float(_DR)
    )  # [16]

    def rope_maps(pos):
        ang = pos.astype(np.float32)[..., None] * inv  # [B, N, 16]
        cos = np.repeat(np.cos(ang), 2, axis=-1)  # [B, N, 32]
        sin = np.repeat(np.sin(ang), 2, axis=-1)
        return cos.transpose(0, 2, 1), sin.transpose(0, 2, 1)  # [B, 32, N]

    ch, sh = rope_maps(np.asarray(pos_h))
    cw, sw = rope_maps(np.asarray(pos_w))
    cos64 = np.concatenate([ch, cw], axis=1)  # [B, 64, N]
    sin64 = np.concatenate([sh, sw], axis=1)
    sign = np.where((np.arange(64) % 32) < 16, -1.0, 1.0).astype(np.float32)
    ssin64 = sin64 * sign[None, :, None]
    cosm = np.tile(cos64, (1, 2, 1)).astype(bf16)  # [B, 128, N]
    ssinm = np.tile(ssin64, (1, 2, 1)).astype(bf16)

    xb = np.asarray(x).astype(bf16)
    wqb = np.ascontiguousarray(np.asarray(W_qkv).astype(bf16))
    wpb = np.ascontiguousarray(np.asarray(W_proj).astype(bf16))

    in_maps = []
    for i in range(_NCORES):
        lo, hi = i * _BL, (i + 1) * _BL
        m = {
            "x": np.ascontiguousarray(xb[lo:hi].reshape(_BL * _N, _C)),
            "wq": wqb,
            "wp": wpb,
            "cosm": np.ascontiguousarray(cosm[lo:hi].reshape(_BL * 128, _N)),
            "ssinm": np.ascontiguousarray(ssinm[lo:hi].reshape(_BL * 128, _N)),
        }
        if has_bias:
            bq = np.asarray(b_qkv).astype(bf16)
            m["bqk"] = np.ascontiguousarray(bq[:2 * _C].reshape(1, 2 * _C))
            m["bv"] = np.ascontiguousarray(bq[2 * _C:].reshape(1, _C))
            m["bp"] = np.ascontiguousarray(
                np.asarray(b_proj).astype(bf16).reshape(1, _C)
            )
        in_maps.append(m)
    return in_maps, has_bias


def _ensure_ntff_hook():
    """This image's antenv lacks axon_hooks; recreate it from the boot
    helper so run_bass_kernel_spmd(trace=True) can capture NTFF profiles."""
    import sys
    import types

    if "antenv.axon_hooks" in sys.modules:
        return
    try:
        from trn_agent_boot.trn_boot import _ntff_profile_via_ctypes

        hook = _ntff_profile_via_ctypes("/opt/axon/libaxon_pjrt.so")
    except Exception:
        hook = None
    mod = types.ModuleType("antenv.axon_hooks")
    mod._hook = hook
    mod.get_axon_ntff_profile_hook = lambda: mod._hook
    mod.set_axon_ntff_profile_hook = lambda h: setattr(mod, "_hook", h)
    sys.modules["antenv.axon_hooks"] = mod


def run(x, W_qkv, b_qkv, W_proj, b_proj, pos_h, pos_w, num_heads, **run_kwargs):
    """Build + execute on 8 NeuronCores; returns (output, BassKernelResults)."""
    from concourse.bass_utils import run_bass_kernel_spmd

    if run_kwargs.get("trace"):
        _ensure_ntff_hook()

    assert int(num_heads) == _H
    in_maps, has_bias = _prep_in_maps(
        x, W_qkv, b_qkv, W_proj, b_proj, pos_h, pos_w
    )
    nc = _get_nc(has_bias)
    res = run_bass_kernel_spmd(
        nc, in_maps, core_ids=list(range(_NCORES)), **run_kwargs
    )
    out = np.concatenate(
        [res.results[i]["out"].reshape(_BL, _N, _C) for i in range(_NCORES)],
        axis=0,
    ).astype(np.float32)
    return out, res


def kernel(x, W_qkv, b_qkv, W_proj, b_proj, pos_h, pos_w, num_heads):
    out, _ = run(x, W_qkv, b_qkv, W_proj, b_proj, pos_h, pos_w, num_heads)
    return out


# revision 8
# speedup vs baseline: 1.2463x; 1.2463x over previous
"""Trainium2 Bass kernel for nn_Attention_24704651887034.

Dense ViT-style attention block (B=16, N=1024, C=768, H=12 heads, 2D RoPE),
data-parallel over batch across 8 NeuronCores (2 batch items per core, no
collectives).

v2 — HAM-warm dense-PE schedule:
  * The PE array clock-gates to 1.2 GHz unless continuously busy (~3.4us
    windows).  The kernel therefore emits a single dense PE instruction
    stream: attention steps are interleaved with "filler" matmul closures
    (next item's QKV/V, previous item's proj) so the PE never idles while
    the Scalar engine computes softmax exps.
  * Score matmuls for the two heads of a pair are emitted adjacently on
    row-tiles (0,0)/(64,0) so the 64-contraction matmuls run concurrently
    in the two halves of the PE array.
  * exp is fused over kc-pairs ([128,1024] ACT ops) to halve ACT overhead.
  * Softmax denominators stay in PSUM row 64 ([v|1] ones trick); per
    (pair, q-half) they are inverted with the single-pass DVE
    reciprocal_approx_fast, DMA-broadcast, and fused into the PSUM->SBUF
    evacuation multiply (no separate normalize pass, nothing on the
    critical path at the proj boundary).
"""

from collections import deque

import numpy as np

_B, _N, _C, _H = 16, 1024, 768, 12
_HD, _DR = 64, 32
_ROPE_BASE = 10000.0
_NCORES = 8
_BL = _B // _NCORES  # batch items per core

_NC6 = _C // 128      # 6 contraction chunks

_nc_cache = {}


def _split_excess_waits(nc, max_waits=1):
    """Walrus in this toolchain accepts at most one sync-wait command per
    instruction; Tile's tail drain (and occasionally the scheduler) emits
    more. Split the excess onto same-engine NOPs inserted just before."""
    from concourse import mybir

    for f in nc.m.functions:
        for blk in f.blocks:
            insts = blk.instructions
            i = 0
            while i < len(insts):
                ins = insts[i]
                si = getattr(ins, "sync_info", None)
                if si is not None and len(si.on_wait) > max_waits:
                    excess = si.on_wait[max_waits:]
                    ins.sync_info = mybir.SyncInfo(
                        on_wait=list(si.on_wait[:max_waits]),
                        on_update=list(si.on_update),
                    )
                    for j, w in enumerate(excess):
                        nop = mybir.InstNoOp(
                            name=f"{ins.name}-sw{j}", engine=ins.engine
                        )
                        nop.sync_info = mybir.SyncInfo(on_wait=[w], on_update=[])
                        insts.insert(i, nop)
                        i += 1
                i += 1
    return nc


def _build(has_bias):
    from contextlib import ExitStack

    import concourse.bass as bass
    import concourse.tile as tile
    from concourse import mybir

    BF = mybir.dt.bfloat16
    F32 = mybir.dt.float32
    Exp = mybir.ActivationFunctionType.Exp
    N, C, H = _N, _C, _H
    BL = _BL

    nc = bass.Bass("TRN2", target_bir_lowering=False, debug=False)
    x_d = nc.dram_tensor("x", [BL * N, C], BF, kind="ExternalInput").ap()
    wq_d = nc.dram_tensor("wq", [C, 3 * C], BF, kind="ExternalInput").ap()
    wp_d = nc.dram_tensor("wp", [C, C], BF, kind="ExternalInput").ap()
    cos_d = nc.dram_tensor("cosm", [BL * 128, N], BF, kind="ExternalInput").ap()
    sin_d = nc.dram_tensor("ssinm", [BL * 128, N], BF, kind="ExternalInput").ap()
    if has_bias:
        bqk_d = nc.dram_tensor("bqk", [1, 2 * C], BF, kind="ExternalInput").ap()
        bv_d = nc.dram_tensor("bv", [1, C], BF, kind="ExternalInput").ap()
        bp_d = nc.dram_tensor("bp", [1, C], BF, kind="ExternalInput").ap()
    out_d = nc.dram_tensor("out", [BL * N, C], F32, kind="ExternalOutput").ap()

    SH_MASK = [(i + 16) % 32 for i in range(32)]
    F_ORDER = [0, 6, 1, 7, 2, 8, 3, 9, 4, 10, 5, 11]

    with tile.TileContext(nc) as tc, ExitStack() as ctx:
        const = ctx.enter_context(tc.tile_pool(name="const", bufs=1))
        xT_p = ctx.enter_context(tc.tile_pool(name="xT", bufs=2 * _NC6))
        map_p = ctx.enter_context(tc.tile_pool(name="maps", bufs=2))
        raw_p = ctx.enter_context(tc.tile_pool(name="qraw", bufs=3))
        tmp_p = ctx.enter_context(tc.tile_pool(name="rtmp", bufs=3))
        tmp2_p = ctx.enter_context(tc.tile_pool(name="rtmp2", bufs=2))
        qkr_p = ctx.enter_context(tc.tile_pool(name="qkr", bufs=23))
        v_p = ctx.enter_context(tc.tile_pool(name="v", bufs=16))
        pT_p = ctx.enter_context(tc.tile_pool(name="pT", bufs=3))
        ao_p = ctx.enter_context(tc.tile_pool(name="ao", bufs=12))
        s65_p = ctx.enter_context(tc.tile_pool(name="s65", bufs=4))
        rec_p = ctx.enter_context(tc.tile_pool(name="rec", bufs=2))
        bch_p = ctx.enter_context(tc.tile_pool(name="bch", bufs=2))
        ost_p = ctx.enter_context(tc.tile_pool(name="ost", bufs=2))
        mm_ps = ctx.enter_context(tc.tile_pool(name="mmps", bufs=2, space="PSUM"))
        sc_ps = ctx.enter_context(tc.tile_pool(name="scps", bufs=1, space="PSUM"))
        o_ps = ctx.enter_context(tc.tile_pool(name="ops", bufs=2, space="PSUM"))

        # ---- resident constants: weights ----
        wq_t = []
        for c in range(_NC6):
            t = const.tile([128, 3 * C], BF, tag=f"wq{c}")
            nc.sync.dma_start(
                t[:, 0:2 * C], wq_d[c * 128:(c + 1) * 128, 0:2 * C]
            )
            wq_t.append(t)
        for c in range(_NC6):
            nc.sync.dma_start(
                wq_t[c][:, 2 * C:3 * C], wq_d[c * 128:(c + 1) * 128, 2 * C:3 * C]
            )
        wp_t = []
        for c in range(_NC6):
            t = const.tile([128, C], BF, tag=f"wp{c}")
            nc.sync.dma_start(t[:], wp_d[c * 128:(c + 1) * 128, :])
            wp_t.append(t)
        if has_bias:
            bqk_sb = const.tile([1, 2 * C], BF, tag="bqk")
            nc.sync.dma_start(bqk_sb[:], bqk_d[:])
            bv_sb = const.tile([1, C], BF, tag="bv")
            nc.sync.dma_start(bv_sb[:], bv_d[:])
            bp_sb = const.tile([1, C], BF, tag="bp")
            nc.sync.dma_start(bp_sb[:], bp_d[:])
            ones_r = const.tile([1, 512], BF, tag="ones")
            nc.gpsimd.memset(ones_r[:], 1.0)

        # ---- per-item input DMAs, all issued upfront ----
        xT = {}
        cosm = {}
        ssin = {}
        for b in range(BL):
            for c in range(_NC6):
                t = xT_p.tile([128, N], BF, tag="xT", name=f"xT{b}_{c}")
                nc.sync.dma_start(
                    t[:], x_d[b * N:(b + 1) * N, c * 128:(c + 1) * 128],
                    transpose=True,
                )
                xT[b, c] = t
            m = map_p.tile([128, N], BF, tag="cos", name=f"cos{b}")
            nc.sync.dma_start(m[:], cos_d[b * 128:(b + 1) * 128, :])
            cosm[b] = m
            m = map_p.tile([128, N], BF, tag="sin", name=f"sin{b}")
            nc.sync.dma_start(m[:], sin_d[b * 128:(b + 1) * 128, :])
            ssin[b] = m

        qk_r = {}
        v_sb = {}
        ao = {}

        # ---- emitters ----
        def emit_q_half(b, f, t2, raw):
            ps = mm_ps.tile([128, 512], F32, tag="mm", name=f"q{b}_{f}_{t2}")
            for c in range(_NC6):
                nc.tensor.matmul(
                    ps[:],
                    wq_t[c][:, f * 128:(f + 1) * 128],
                    xT[b, c][:, t2 * 512:(t2 + 1) * 512],
                    start=(c == 0),
                    stop=(c == _NC6 - 1 and not has_bias),
                )
            if has_bias:
                nc.tensor.matmul(
                    ps[:],
                    bqk_sb[:, f * 128:(f + 1) * 128],
                    ones_r[:],
                    start=False,
                    stop=True,
                )
            nc.scalar.copy(raw[:, t2 * 512:(t2 + 1) * 512], ps[:])

        def emit_q_rope(b, f, raw):
            r = tmp_p.tile([128, N], BF, tag="rt", name=f"rr{b}_{f}")
            nc.vector.stream_shuffle(r[:], raw[:], SH_MASK)
            tm = tmp2_p.tile([128, N], BF, tag="rm", name=f"rm{b}_{f}")
            nc.vector.tensor_mul(tm[:], r[:], ssin[b][:])
            ro = qkr_p.tile([128, N], BF, tag="qkr", name=f"qkr{b}_{f}")
            nc.gpsimd.tensor_mul(ro[:], raw[:], cosm[b][:])
            nc.vector.tensor_add(ro[:], ro[:], tm[:])
            qk_r[b, f] = ro

        def mk_q_closures(b, f):
            st = {}

            def c1():
                st["raw"] = raw_p.tile([128, N], BF, tag="qraw", name=f"qw{b}_{f}")
                emit_q_half(b, f, 0, st["raw"])

            def c2():
                emit_q_half(b, f, 1, st["raw"])
                emit_q_rope(b, f, st["raw"])

            return [c1, c2]

        def mk_v_closures(b, t8):
            st = {}

            def grp(f0, fw, first):
                vt = st["vt"]
                vt3 = vt.rearrange("p (h w) -> p h w", w=65)
                if first:
                    nc.gpsimd.memset(vt3[:, :, 64:65], 1.0)
                ps = mm_ps.tile([128, 512], F32, tag="mm", name=f"v{b}_{t8}_{f0}")
                for c in range(_NC6):
                    nc.tensor.matmul(
                        ps[:, :fw],
                        xT[b, c][:, t8 * 128:(t8 + 1) * 128],
                        wq_t[c][:, 2 * C + f0:2 * C + f0 + fw],
                        start=(c == 0),
                        stop=(c == _NC6 - 1 and not has_bias),
                    )
                if has_bias:
                    nc.tensor.matmul(
                        ps[:, :fw],
                        ones_r[:, t8 * 128 % 512:t8 * 128 % 512 + 128],
                        bv_sb[:, f0:f0 + fw],
                        start=False,
                        stop=True,
                    )
                nh = fw // 64
                nc.vector.tensor_copy(
                    vt3[:, f0 // 64:f0 // 64 + nh, 0:64],
                    ps[:, :fw].rearrange("p (h w) -> p h w", w=64),
                )

            def c1():
                st["vt"] = v_p.tile([128, H * 65], BF, tag="v", name=f"v{b}_{t8}")
                v_sb[b, t8] = st["vt"]
                grp(0, 512, True)

            def c2():
                grp(512, 256, False)

            return [c1, c2]

        def mk_p_closure(b, t8, nf):
            def c1():
                ps = mm_ps.tile([128, 512], F32, tag="mm", name=f"p{b}_{t8}_{nf}")
                for jj in range(_NC6):
                    nc.tensor.matmul(
                        ps[:, :384],
                        ao[b, jj][:, t8 * 128:(t8 + 1) * 128],
                        wp_t[jj][:, nf * 384:(nf + 1) * 384],
                        start=(jj == 0),
                        stop=(jj == _NC6 - 1 and not has_bias),
                    )
                if has_bias:
                    nc.tensor.matmul(
                        ps[:, :384],
                        ones_r[:, 0:128],
                        bp_sb[:, nf * 384:(nf + 1) * 384],
                        start=False,
                        stop=True,
                    )
                ot = ost_p.tile([128, 384], F32, tag="ost", name=f"ot{b}_{t8}_{nf}")
                nc.vector.tensor_copy(ot[:], ps[:, :384])
                nc.sync.dma_start(
                    out_d[b * N + t8 * 128:b * N + (t8 + 1) * 128,
                          nf * 384:(nf + 1) * 384],
                    ot[:],
                )

            return [c1]

        # ---- filler machinery ----
        fillers = deque()
        fstate = {"acc": 0.0, "ratio": 0.0}

        def set_fill_ratio(n_points):
            fstate["ratio"] = (len(fillers) / n_points) if n_points else 0.0
            fstate["acc"] = 0.0

        def fill():
            fstate["acc"] += fstate["ratio"]
            while fstate["acc"] >= 1.0 and fillers:
                fillers.popleft()()
                fstate["acc"] -= 1.0

        def flush_fillers():
            while fillers:
                fillers.popleft()()

        # ---- attention pair emitter ----
        def emit_att_pair(b, j):
            qT = qk_r[b, j]
            kT = qk_r[b, 6 + j]
            if (b, j) not in ao:
                ao[b, j] = ao_p.tile([128, N], BF, tag="ao", name=f"ao{b}_{j}")
            for qh in range(2):
                qsl = slice(qh * 512, (qh + 1) * 512)
                o_t = [
                    o_ps.tile([65, 512], F32, tag="o", name=f"o{b}_{j}_{qh}_{h2}")
                    for h2 in range(2)
                ]
                pts = [None] * 4
                for i in range(5):
                    if i < 4:
                        # one [128,2048] tile: cols [h2*1024 + u*512] hold
                        # scores(kc=2i+u) for head pair member h2; the four
                        # matmuls are emitted h2-adjacent so the 64-row tiles
                        # (0,0)/(64,0) overlap in the array.
                        sc = sc_ps.tile([128, 2048], F32, tag="sc",
                                        name=f"s{b}_{j}_{qh}_{i}")
                        for u in range(2):
                            kc = 2 * i + u
                            for h2 in range(2):
                                half = h2 * 64
                                nc.tensor.matmul(
                                    sc[:, h2 * 1024 + u * 512:
                                       h2 * 1024 + (u + 1) * 512],
                                    kT[half:half + 64, kc * 128:(kc + 1) * 128],
                                    qT[half:half + 64, qsl],
                                    start=True,
                                    stop=True,
                                )
                        pt = pT_p.tile([128, 2048], BF, tag="pt",
                                       name=f"pt{b}_{j}_{qh}_{i}")
                        nc.scalar.activation(pt[:], sc[:], Exp, scale=0.125)
                        pts[i] = pt
                    if i >= 1:
                        for u in range(2):
                            kd = 2 * (i - 1) + u
                            for h2 in range(2):
                                h = 2 * j + h2
                                vt3 = v_sb[b, kd].rearrange("p (h w) -> p h w", w=65)
                                nc.tensor.matmul(
                                    o_t[h2][:],
                                    vt3[:, h, 0:65],
                                    pts[i - 1][:, h2 * 1024 + u * 512:
                                               h2 * 1024 + (u + 1) * 512],
                                    start=(kd == 0),
                                    stop=(kd == 7),
                                )
                    fill()
                # evacuate [v-rows | denominator] to SBUF, freeing the PSUM
                # bank with a single DVE copy; the reciprocal+broadcast chain
                # then runs off the critical path.
                s65s = []
                for h2 in range(2):
                    nm = f"{b}_{j}_{qh}_{h2}"
                    s65 = s65_p.tile([65, 512], BF, tag="s65", name=f"e{nm}")
                    nc.vector.tensor_copy(s65[:], o_t[h2][:])
                    s65s.append(s65)
                d32 = rec_p.tile([32, 32], BF, tag="d32",
                                 name=f"d{b}_{j}_{qh}")
                for h2 in range(2):
                    nc.sync.dma_start(
                        d32[h2 * 16:(h2 + 1) * 16, :],
                        s65s[h2][64:65, :].rearrange("p (a w) -> p a w", a=16),
                    )
                d32r = rec_p.tile([32, 32], F32, tag="d32r",
                                  name=f"r{b}_{j}_{qh}")
                nc.vector.reciprocal(d32r[:], d32[:])
                rr2 = rec_p.tile([2, 512], F32, tag="rr2",
                                 name=f"w{b}_{j}_{qh}")
                nc.sync.dma_start(
                    rr2[:].rearrange("p (a w) -> p a w", a=16), d32r[:]
                )
                fill()
                for h2 in range(2):
                    half = h2 * 64
                    nm = f"{b}_{j}_{qh}_{h2}"
                    bch = bch_p.tile([64, 512], F32, tag="bch", name=f"bc{nm}")
                    nc.sync.dma_start(
                        bch[:],
                        rr2[h2:h2 + 1, :]
                        .rearrange("p (u n) -> p u n", u=1)
                        .broadcast_to((1, 64, 512)),
                    )
                    nc.vector.tensor_mul(
                        ao[b, j][half:half + 64, qsl], s65s[h2][0:64, :], bch[:]
                    )
                fill()

        # ================= main schedule =================
        # item 0: QKV + V inline (PE-dense, ACT idle -> evac on ACT)
        for f in F_ORDER:
            raw = raw_p.tile([128, N], BF, tag="qraw", name=f"qw0_{f}")
            emit_q_half(0, f, 0, raw)
            emit_q_half(0, f, 1, raw)
            emit_q_rope(0, f, raw)
        for t8 in range(8):
            for c in mk_v_closures(0, t8):
                c()

        # attention item 0, filled with item-1 QKV + V
        for f in F_ORDER:
            fillers.extend(mk_q_closures(1, f))
        for t8 in range(8):
            fillers.extend(mk_v_closures(1, t8))
        set_fill_ratio(6 * 2 * 6)
        for j in range(_NC6):
            emit_att_pair(0, j)
        flush_fillers()

        # attention item 1, filled with item-0 proj
        for t8 in range(8):
            for nf in range(2):
                fillers.extend(mk_p_closure(0, t8, nf))
        set_fill_ratio(6 * 2 * 6)
        for j in range(_NC6):
            emit_att_pair(1, j)
        flush_fillers()

        # proj item 1 inline
        for t8 in range(8):
            for nf in range(2):
                for c in mk_p_closure(1, t8, nf):
                    c()
    return _split_excess_waits(nc)


def _get_nc(has_bias):
    if has_bias not in _nc_cache:
        _nc_cache[has_bias] = _build(has_bias)
    return _nc_cache[has_bias]


def _prep_in_maps(x, W_qkv, b_qkv, W_proj, b_proj, pos_h, pos_w):
    import ml_dtypes

    bf16 = ml_dtypes.bfloat16
    has_bias = bool(np.any(b_qkv)) or bool(np.any(b_proj))

    inv = 1.0 / _ROPE_BASE ** (
        np.arange(0, _DR, 2, dtype=np.float32) / float(_DR)
    )  # [16]

    def rope_maps(pos):
        ang = pos.astype(np.float32)[..., None] * inv  # [B, N, 16]
        cos = np.repeat(np.cos(ang), 2, axis=-1)  # [B, N, 32]
        sin = np.repeat(np.sin(ang), 2, axis=-1)
        return cos.transpose(0, 2, 1), sin.transpose(0, 2, 1)  # [B, 32, N]

    ch, sh = rope_maps(np.asarray(pos_h))
    cw, sw = rope_maps(np.asarray(pos_w))
    cos64 = np.concatenate([ch, cw], axis=1)  # [B, 64, N]
    sin64 = np.concatenate([sh, sw], axis=1)
    sign = np.where((np.arange(64) % 32) < 16, -1.0, 1.0).astype(np.float32)
    ssin64 = sin64 * sign[None, :, None]
    cosm = np.tile(cos64, (1, 2, 1)).astype(bf16)  # [B, 128, N]
    ssinm = np.tile(ssin64, (1, 2, 1)).astype(bf16)

    xb = np.asarray(x).astype(bf16)
    wqb = np.ascontiguousarray(np.asarray(W_qkv).astype(bf16))
    wpb = np.ascontiguousarray(np.asarray(W_proj).astype(bf16))

    in_maps = []
    for i in range(_NCORES):
        lo, hi = i * _BL, (i + 1) * _BL
        m = {
            "x": np.ascontiguousarray(xb[lo:hi].reshape(_BL * _N, _C)),
            "wq": wqb,
            "wp": wpb,
            "cosm": np.ascontiguousarray(cosm[lo:hi].reshape(_BL * 128, _N)),
            "ssinm": np.ascontiguousarray(ssinm[lo:hi].reshape(_BL * 128, _N)),
        }
        if has_bias:
            bq = np.asarray(b_qkv).astype(bf16)
            m["bqk"] = np.ascontiguousarray(bq[:2 * _C].reshape(1, 2 * _C))
            m["bv"] = np.ascontiguousarray(bq[2 * _C:].reshape(1, _C))
            m["bp"] = np.ascontiguousarray(
                np.asarray(b_proj).astype(bf16).reshape(1, _C)
            )
        in_maps.append(m)
    return in_maps, has_bias


def _ensure_ntff_hook():
    """This image's antenv lacks axon_hooks; recreate it from the boot
    helper so run_bass_kernel_spmd(trace=True) can capture NTFF profiles."""
    import sys
    import types

    if "antenv.axon_hooks" in sys.modules:
        return
    try:
        from trn_agent_boot.trn_boot import _ntff_profile_via_ctypes

        hook = _ntff_profile_via_ctypes("/opt/axon/libaxon_pjrt.so")
    except Exception:
        hook = None
    mod = types.ModuleType("antenv.axon_hooks")
    mod._hook = hook
    mod.get_axon_ntff_profile_hook = lambda: mod._hook
    mod.set_axon_ntff_profile_hook = lambda h: setattr(mod, "_hook", h)
    sys.modules["antenv.axon_hooks"] = mod


def run(x, W_qkv, b_qkv, W_proj, b_proj, pos_h, pos_w, num_heads, **run_kwargs):
    """Build + execute on 8 NeuronCores; returns (output, BassKernelResults)."""
    from concourse.bass_utils import run_bass_kernel_spmd

    if run_kwargs.get("trace"):
        _ensure_ntff_hook()

    assert int(num_heads) == _H
    in_maps, has_bias = _prep_in_maps(
        x, W_qkv, b_qkv, W_proj, b_proj, pos_h, pos_w
    )
    nc = _get_nc(has_bias)
    res = run_bass_kernel_spmd(
        nc, in_maps, core_ids=list(range(_NCORES)), **run_kwargs
    )
    out = np.concatenate(
        [res.results[i]["out"].reshape(_BL, _N, _C) for i in range(_NCORES)],
        axis=0,
    ).astype(np.float32)
    return out, res


def kernel(x, W_qkv, b_qkv, W_proj, b_proj, pos_h, pos_w, num_heads):
    out, _ = run(x, W_qkv, b_qkv, W_proj, b_proj, pos_h, pos_w, num_heads)
    return out


# revision 15
# speedup vs baseline: 1.2543x; 1.0064x over previous
"""Trainium2 Bass kernel for nn_Attention_24704651887034.

Dense ViT-style attention block (B=16, N=1024, C=768, H=12 heads, 2D RoPE),
data-parallel over batch across 8 NeuronCores (2 batch items per core, no
collectives).

v2 — HAM-warm dense-PE schedule:
  * The PE array clock-gates to 1.2 GHz unless continuously busy (~3.4us
    windows).  The kernel therefore emits a single dense PE instruction
    stream: attention steps are interleaved with "filler" matmul closures
    (next item's QKV/V, previous item's proj) so the PE never idles while
    the Scalar engine computes softmax exps.
  * Score matmuls for the two heads of a pair are emitted adjacently on
    row-tiles (0,0)/(64,0) so the 64-contraction matmuls run concurrently
    in the two halves of the PE array.
  * exp is fused over kc-pairs ([128,1024] ACT ops) to halve ACT overhead.
  * Softmax denominators stay in PSUM row 64 ([v|1] ones trick); per
    (pair, q-half) they are inverted with the single-pass DVE
    reciprocal_approx_fast, DMA-broadcast, and fused into the PSUM->SBUF
    evacuation multiply (no separate normalize pass, nothing on the
    critical path at the proj boundary).
"""

from collections import deque

import numpy as np

_B, _N, _C, _H = 16, 1024, 768, 12
_HD, _DR = 64, 32
_ROPE_BASE = 10000.0
_NCORES = 8
_BL = _B // _NCORES  # batch items per core

_NC6 = _C // 128      # 6 contraction chunks

_nc_cache = {}


def _split_excess_waits(nc, max_waits=1):
    """Walrus in this toolchain accepts at most one sync-wait command per
    instruction; Tile's tail drain (and occasionally the scheduler) emits
    more. Split the excess onto same-engine NOPs inserted just before."""
    from concourse import mybir

    for f in nc.m.functions:
        for blk in f.blocks:
            insts = blk.instructions
            i = 0
            while i < len(insts):
                ins = insts[i]
                si = getattr(ins, "sync_info", None)
                if si is not None and len(si.on_wait) > max_waits:
                    excess = si.on_wait[max_waits:]
                    ins.sync_info = mybir.SyncInfo(
                        on_wait=list(si.on_wait[:max_waits]),
                        on_update=list(si.on_update),
                    )
                    for j, w in enumerate(excess):
                        nop = mybir.InstNoOp(
                            name=f"{ins.name}-sw{j}", engine=ins.engine
                        )
                        nop.sync_info = mybir.SyncInfo(on_wait=[w], on_update=[])
                        insts.insert(i, nop)
                        i += 1
                i += 1
    return nc


def _build(has_bias):
    from contextlib import ExitStack

    import concourse.bass as bass
    import concourse.tile as tile
    from concourse import mybir

    BF = mybir.dt.bfloat16
    F32 = mybir.dt.float32
    Exp = mybir.ActivationFunctionType.Exp
    N, C, H = _N, _C, _H
    BL = _BL

    nc = bass.Bass("TRN2", target_bir_lowering=False, debug=False)
    x_d = nc.dram_tensor("x", [BL * N, C], BF, kind="ExternalInput").ap()
    wq_d = nc.dram_tensor("wq", [C, 3 * C], BF, kind="ExternalInput").ap()
    wp_d = nc.dram_tensor("wp", [C, C], BF, kind="ExternalInput").ap()
    cos_d = nc.dram_tensor("cosm", [BL * 128, N], BF, kind="ExternalInput").ap()
    sin_d = nc.dram_tensor("ssinm", [BL * 128, N], BF, kind="ExternalInput").ap()
    if has_bias:
        bqk_d = nc.dram_tensor("bqk", [1, 2 * C], BF, kind="ExternalInput").ap()
        bv_d = nc.dram_tensor("bv", [1, C], BF, kind="ExternalInput").ap()
        bp_d = nc.dram_tensor("bp", [1, C], BF, kind="ExternalInput").ap()
    out_d = nc.dram_tensor("out", [BL * N, C], BF, kind="ExternalOutput").ap()

    SH_MASK = [(i + 16) % 32 for i in range(32)]
    F_ORDER = [0, 6, 1, 7, 2, 8, 3, 9, 4, 10, 5, 11]

    with tile.TileContext(nc) as tc, ExitStack() as ctx:
        const = ctx.enter_context(tc.tile_pool(name="const", bufs=1))
        xT_p = ctx.enter_context(tc.tile_pool(name="xT", bufs=2 * _NC6))
        map_p = ctx.enter_context(tc.tile_pool(name="maps", bufs=2))
        raw_p = ctx.enter_context(tc.tile_pool(name="qraw", bufs=3))
        tmp_p = ctx.enter_context(tc.tile_pool(name="rtmp", bufs=2))
        tmp2_p = ctx.enter_context(tc.tile_pool(name="rtmp2", bufs=3))
        qkr_p = ctx.enter_context(tc.tile_pool(name="qkr", bufs=22))
        v_p = ctx.enter_context(tc.tile_pool(name="v", bufs=16))
        pT_p = ctx.enter_context(tc.tile_pool(name="pT", bufs=3))
        ao_p = ctx.enter_context(tc.tile_pool(name="ao", bufs=12))
        s65_p = ctx.enter_context(tc.tile_pool(name="s65", bufs=7))
        rec_p = ctx.enter_context(tc.tile_pool(name="rec", bufs=3))
        rr2_p = ctx.enter_context(tc.tile_pool(name="rr2", bufs=2))
        bch_p = ctx.enter_context(tc.tile_pool(name="bch", bufs=3))
        ost_p = ctx.enter_context(tc.tile_pool(name="ost", bufs=2))
        mm_ps = ctx.enter_context(tc.tile_pool(name="mmps", bufs=2, space="PSUM"))
        sc_ps = ctx.enter_context(tc.tile_pool(name="scps", bufs=1, space="PSUM"))
        o_ps = ctx.enter_context(tc.tile_pool(name="ops", bufs=2, space="PSUM"))

        # ---- resident constants: weights ----
        wq_t = []
        for c in range(_NC6):
            t = const.tile([128, 3 * C], BF, tag=f"wq{c}")
            nc.sync.dma_start(
                t[:, 0:2 * C], wq_d[c * 128:(c + 1) * 128, 0:2 * C]
            )
            wq_t.append(t)

        if has_bias:
            bqk_sb = const.tile([1, 2 * C], BF, tag="bqk")
            nc.sync.dma_start(bqk_sb[:], bqk_d[:])
            bv_sb = const.tile([1, C], BF, tag="bv")
            nc.sync.dma_start(bv_sb[:], bv_d[:])
            bp_sb = const.tile([1, C], BF, tag="bp")
            nc.sync.dma_start(bp_sb[:], bp_d[:])
            ones_r = const.tile([1, 512], BF, tag="ones")
            nc.gpsimd.memset(ones_r[:], 1.0)

        # ---- per-item input DMAs, ordered by first use ----
        xT = {}
        cosm = {}
        ssin = {}

        def emit_item_inputs(b):
            for c in range(_NC6):
                t = xT_p.tile([128, N], BF, tag="xT", name=f"xT{b}_{c}")
                nc.sync.dma_start(
                    t[:], x_d[b * N:(b + 1) * N, c * 128:(c + 1) * 128],
                    transpose=True,
                )
                xT[b, c] = t
            m = map_p.tile([128, N], BF, tag="cos", name=f"cos{b}")
            nc.sync.dma_start(m[:], cos_d[b * 128:(b + 1) * 128, :])
            cosm[b] = m
            m = map_p.tile([128, N], BF, tag="sin", name=f"sin{b}")
            nc.sync.dma_start(m[:], sin_d[b * 128:(b + 1) * 128, :])
            ssin[b] = m

        emit_item_inputs(0)
        for c in range(_NC6):
            nc.sync.dma_start(
                wq_t[c][:, 2 * C:3 * C], wq_d[c * 128:(c + 1) * 128, 2 * C:3 * C]
            )
        wp_t = []
        for c in range(_NC6):
            t = const.tile([128, C], BF, tag=f"wp{c}")
            nc.sync.dma_start(t[:], wp_d[c * 128:(c + 1) * 128, :])
            wp_t.append(t)
        emit_item_inputs(1)

        qk_r = {}
        v_sb = {}
        ao = {}

        # ---- emitters ----
        def emit_q_half(b, f, t2, raw):
            ps = mm_ps.tile([128, 512], F32, tag="mm", name=f"q{b}_{f}_{t2}")
            for c in range(_NC6):
                nc.tensor.matmul(
                    ps[:],
                    wq_t[c][:, f * 128:(f + 1) * 128],
                    xT[b, c][:, t2 * 512:(t2 + 1) * 512],
                    start=(c == 0),
                    stop=(c == _NC6 - 1 and not has_bias),
                )
            if has_bias:
                nc.tensor.matmul(
                    ps[:],
                    bqk_sb[:, f * 128:(f + 1) * 128],
                    ones_r[:],
                    start=False,
                    stop=True,
                )
            nc.scalar.copy(raw[:, t2 * 512:(t2 + 1) * 512], ps[:])

        def emit_q_rope(b, f, raw, inline=False):
            r = tmp_p.tile([128, N], BF, tag="rt", name=f"rr{b}_{f}")
            nc.vector.stream_shuffle(r[:], raw[:], SH_MASK)
            tm = tmp2_p.tile([128, N], BF, tag="rm", name=f"rm{b}_{f}")
            nc.vector.tensor_mul(tm[:], r[:], ssin[b][:])
            ro = qkr_p.tile([128, N], BF, tag="qkr", name=f"qkr{b}_{f}")
            nc.gpsimd.tensor_mul(ro[:], raw[:], cosm[b][:])
            qk_r[b, f] = ro

            def add():
                nc.vector.tensor_add(ro[:], ro[:], tm[:])

            if inline:
                add()
            else:
                defer(3, add, key=("qadd", b, f))

        def mk_q_closures(b, f):
            st = {}

            def c1():
                st["raw"] = raw_p.tile([128, N], BF, tag="qraw", name=f"qw{b}_{f}")
                emit_q_half(b, f, 0, st["raw"])

            def c2():
                emit_q_half(b, f, 1, st["raw"])
                emit_q_rope(b, f, st["raw"])

            return [(("q", b, f), c1), (("q", b, f), c2)]

        def mk_v_closures(b, t8):
            st = {}

            def grp(f0, fw, first):
                vt = st["vt"]
                vt3 = vt.rearrange("p (h w) -> p h w", w=65)
                if first:
                    nc.gpsimd.memset(vt3[:, :, 64:65], 1.0)
                ps = mm_ps.tile([128, 512], F32, tag="mm", name=f"v{b}_{t8}_{f0}")
                for c in range(_NC6):
                    nc.tensor.matmul(
                        ps[:, :fw],
                        xT[b, c][:, t8 * 128:(t8 + 1) * 128],
                        wq_t[c][:, 2 * C + f0:2 * C + f0 + fw],
                        start=(c == 0),
                        stop=(c == _NC6 - 1 and not has_bias),
                    )
                if has_bias:
                    nc.tensor.matmul(
                        ps[:, :fw],
                        ones_r[:, t8 * 128 % 512:t8 * 128 % 512 + 128],
                        bv_sb[:, f0:f0 + fw],
                        start=False,
                        stop=True,
                    )
                nh = fw // 64
                nc.vector.tensor_copy(
                    vt3[:, f0 // 64:f0 // 64 + nh, 0:64],
                    ps[:, :fw].rearrange("p (h w) -> p h w", w=64),
                )

            def c1():
                st["vt"] = v_p.tile([128, H * 65], BF, tag="v", name=f"v{b}_{t8}")
                v_sb[b, t8] = st["vt"]
                grp(0, 512, True)

            def c2():
                grp(512, 256, False)

            return [(("v", b, t8), c1), (("v", b, t8), c2)]

        def mk_p_closure(b, t8, nf):
            def c1():
                ps = mm_ps.tile([128, 512], F32, tag="mm", name=f"p{b}_{t8}_{nf}")
                for jj in range(_NC6):
                    nc.tensor.matmul(
                        ps[:, :384],
                        ao[b, jj][:, t8 * 128:(t8 + 1) * 128],
                        wp_t[jj][:, nf * 384:(nf + 1) * 384],
                        start=(jj == 0),
                        stop=(jj == _NC6 - 1 and not has_bias),
                    )
                if has_bias:
                    nc.tensor.matmul(
                        ps[:, :384],
                        ones_r[:, 0:128],
                        bp_sb[:, nf * 384:(nf + 1) * 384],
                        start=False,
                        stop=True,
                    )
                ot = ost_p.tile([128, 384], BF, tag="ost", name=f"ot{b}_{t8}_{nf}")
                nc.vector.tensor_copy(ot[:], ps[:, :384])
                nc.sync.dma_start(
                    out_d[b * N + t8 * 128:b * N + (t8 + 1) * 128,
                          nf * 384:(nf + 1) * 384],
                    ot[:],
                )

            return [(("p", b, t8, nf), c1)]

        # ---- filler + deferred-emission machinery ----
        # fillers: (key, closure) of PE work pulled between attention steps.
        # deferred: (due_point, closure) of non-PE tail work (rope adds,
        # reciprocal chains) emitted a few fill-points late so their deps
        # are ready when they enter their engine FIFO (no head-of-line
        # blocking).
        fillers = deque()
        deferred = []
        fstate = {"acc": 0.0, "ratio": 0.0, "pt": 0}

        def set_fill_ratio(n_points):
            fstate["ratio"] = (len(fillers) / n_points) if n_points else 0.0
            fstate["acc"] = 0.0

        def defer(delay, fn, key=None):
            deferred.append((fstate["pt"] + delay, fn, key))

        def force_defer(keys):
            for item in [x for x in deferred]:
                if item[2] in keys:
                    deferred.remove(item)
                    item[1]()

        def fill():
            fstate["pt"] += 1
            for item in [x for x in deferred]:
                if item[0] <= fstate["pt"]:
                    deferred.remove(item)
                    item[1]()
            fstate["acc"] += fstate["ratio"]
            while fstate["acc"] >= 1.0 and fillers:
                fillers.popleft()[1]()
                fstate["acc"] -= 1.0

        def force_fill(keys):
            for item in [x for x in fillers]:
                if item[0] in keys:
                    fillers.remove(item)
                    item[1]()

        def flush_fillers():
            while fillers:
                fillers.popleft()[1]()
            # deferred closures can enqueue more (staged chains): drain fully
            while deferred:
                deferred.sort(key=lambda x: x[0])
                item = deferred.pop(0)
                item[1]()

        # ---- attention pair emitter ----
        def emit_att_pair(b, j):
            force_defer({("qadd", b, j), ("qadd", b, 6 + j)})
            qT = qk_r[b, j]
            kT = qk_r[b, 6 + j]
            if (b, j) not in ao:
                ao[b, j] = ao_p.tile([128, N], BF, tag="ao", name=f"ao{b}_{j}")
            for qh in range(2):
                qsl = slice(qh * 512, (qh + 1) * 512)
                o_t = [
                    o_ps.tile([65, 512], F32, tag="o", name=f"o{b}_{j}_{qh}_{h2}")
                    for h2 in range(2)
                ]
                pts = [None] * 4
                for i in range(5):
                    if i < 4:
                        # one [128,2048] tile: cols [h2*1024 + u*512] hold
                        # scores(kc=2i+u) for head pair member h2; the four
                        # matmuls are emitted h2-adjacent so the 64-row tiles
                        # (0,0)/(64,0) overlap in the array.
                        sc = sc_ps.tile([128, 2048], F32, tag="sc",
                                        name=f"s{b}_{j}_{qh}_{i}")
                        for u in range(2):
                            kc = 2 * i + u
                            for h2 in range(2):
                                half = h2 * 64
                                nc.tensor.matmul(
                                    sc[:, h2 * 1024 + u * 512:
                                       h2 * 1024 + (u + 1) * 512],
                                    kT[half:half + 64, kc * 128:(kc + 1) * 128],
                                    qT[half:half + 64, qsl],
                                    start=True,
                                    stop=True,
                                )
                        pt = pT_p.tile([128, 2048], BF, tag="pt",
                                       name=f"pt{b}_{j}_{qh}_{i}")
                        nc.scalar.activation(pt[:], sc[:], Exp, scale=0.125)
                        pts[i] = pt
                    if i >= 1:
                        for u in range(2):
                            kd = 2 * (i - 1) + u
                            for h2 in range(2):
                                h = 2 * j + h2
                                vt3 = v_sb[b, kd].rearrange("p (h w) -> p h w", w=65)
                                nc.tensor.matmul(
                                    o_t[h2][:],
                                    vt3[:, h, 0:65],
                                    pts[i - 1][:, h2 * 1024 + u * 512:
                                               h2 * 1024 + (u + 1) * 512],
                                    start=(kd == 0),
                                    stop=(kd == 7),
                                )
                    fill()
                # evacuate [v-rows | denominator] to SBUF, freeing the PSUM
                # banks with single DVE copies; the reciprocal/broadcast/
                # normalize chain is emitted in deferred stages so that by
                # the time each op enters its engine FIFO its inputs are
                # already resident (no cross-engine head-of-line blocking).
                s65s = []
                for h2 in range(2):
                    nm = f"{b}_{j}_{qh}_{h2}"
                    s65 = s65_p.tile([65, 512], BF, tag="s65", name=f"e{nm}")
                    nc.vector.tensor_copy(s65[:], o_t[h2][:])
                    s65s.append(s65)
                d32 = rec_p.tile([32, 32], BF, tag="d32",
                                 name=f"d{b}_{j}_{qh}")
                for h2 in range(2):
                    nc.gpsimd.dma_start(
                        d32[h2 * 16:(h2 + 1) * 16, :],
                        s65s[h2][64:65, :].rearrange("p (a w) -> p a w", a=16),
                    )

                def mk_stage2(b=b, j=j, qh=qh, qsl=qsl, s65s=s65s, d32=d32):
                    d32r = rec_p.tile([32, 32], F32, tag="d32r",
                                      name=f"r{b}_{j}_{qh}")
                    nc.vector.reciprocal(d32r[:], d32[:])
                    rr2 = rr2_p.tile([2, 512], F32, tag="rr2",
                                     name=f"w{b}_{j}_{qh}")
                    nc.sync.dma_start(
                        rr2[:].rearrange("p (a w) -> p a w", a=16), d32r[:]
                    )

                    def mk_stage3():
                        bchs = []
                        for h2 in range(2):
                            nm = f"{b}_{j}_{qh}_{h2}"
                            bch = bch_p.tile([64, 512], F32, tag="bch",
                                             name=f"bc{nm}")
                            nc.sync.dma_start(
                                bch[:],
                                rr2[h2:h2 + 1, :]
                                .rearrange("p (u n) -> p u n", u=1)
                                .broadcast_to((1, 64, 512)),
                            )
                            bchs.append(bch)

                        def mk_stage4():
                            for h2 in range(2):
                                nc.vector.tensor_mul(
                                    ao[b, j][h2 * 64:h2 * 64 + 64, qsl],
                                    s65s[h2][0:64, :], bchs[h2][:],
                                )

                        defer(7, mk_stage4)

                    defer(7, mk_stage3)

                defer(7, mk_stage2)
                fill()
                fill()

        # ================= main schedule =================
        # item 0: QKV + V inline (PE-dense, ACT idle -> evac on ACT)
        for f in F_ORDER:
            raw = raw_p.tile([128, N], BF, tag="qraw", name=f"qw0_{f}")
            emit_q_half(0, f, 0, raw)
            emit_q_half(0, f, 1, raw)
            emit_q_rope(0, f, raw, inline=True)
        for t8 in range(8):
            for _, c in mk_v_closures(0, t8):
                c()

        # attention item 0: filled with item-1 V + early QKV chunks
        for f in [0, 6, 1, 7, 2, 8]:
            fillers.extend(mk_q_closures(1, f))
        for t8 in range(8):
            fillers.extend(mk_v_closures(1, t8))
        set_fill_ratio(7 * 12)
        for j in range(_NC6):
            emit_att_pair(0, j)
        while fillers:
            fillers.popleft()[1]()

        # attention item 1: filled with item-1 late QKV chunks + item-0 proj
        for f in [3, 9, 4, 10, 5, 11]:
            fillers.extend(mk_q_closures(1, f))
        for t8 in range(8):
            for nf in range(2):
                fillers.extend(mk_p_closure(0, t8, nf))
        set_fill_ratio(7 * 12)
        for j in range(_NC6):
            if j >= 2:
                force_fill({("q", 1, j), ("q", 1, 6 + j)})
            emit_att_pair(1, j)
        flush_fillers()

        # proj item 1 inline
        for t8 in range(8):
            for nf in range(2):
                for _, c in mk_p_closure(1, t8, nf):
                    c()
    return _split_excess_waits(nc)


def _get_nc(has_bias):
    if has_bias not in _nc_cache:
        _nc_cache[has_bias] = _build(has_bias)
    return _nc_cache[has_bias]


def _prep_in_maps(x, W_qkv, b_qkv, W_proj, b_proj, pos_h, pos_w):
    import ml_dtypes

    bf16 = ml_dtypes.bfloat16
    has_bias = bool(np.any(b_qkv)) or bool(np.any(b_proj))

    inv = 1.0 / _ROPE_BASE ** (
        np.arange(0, _DR, 2, dtype=np.float32) / float(_DR)
    )  # [16]

    def rope_maps(pos):
        ang = pos.astype(np.float32)[..., None] * inv  # [B, N, 16]
        cos = np.repeat(np.cos(ang), 2, axis=-1)  # [B, N, 32]
        sin = np.repeat(np.sin(ang), 2, axis=-1)
        return cos.transpose(0, 2, 1), sin.transpose(0, 2, 1)  # [B, 32, N]

    ch, sh = rope_maps(np.asarray(pos_h))
    cw, sw = rope_maps(np.asarray(pos_w))
    cos64 = np.concatenate([ch, cw], axis=1)  # [B, 64, N]
    sin64 = np.concatenate([sh, sw], axis=1)
    sign = np.where((np.arange(64) % 32) < 16, -1.0, 1.0).astype(np.float32)
    ssin64 = sin64 * sign[None, :, None]
    cosm = np.tile(cos64, (1, 2, 1)).astype(bf16)  # [B, 128, N]
    ssinm = np.tile(ssin64, (1, 2, 1)).astype(bf16)

    xb = np.asarray(x).astype(bf16)
    wqb = np.ascontiguousarray(np.asarray(W_qkv).astype(bf16))
    wpb = np.ascontiguousarray(np.asarray(W_proj).astype(bf16))

    in_maps = []
    for i in range(_NCORES):
        lo, hi = i * _BL, (i + 1) * _BL
        m = {
            "x": np.ascontiguousarray(xb[lo:hi].reshape(_BL * _N, _C)),
            "wq": wqb,
            "wp": wpb,
            "cosm": np.ascontiguousarray(cosm[lo:hi].reshape(_BL * 128, _N)),
            "ssinm": np.ascontiguousarray(ssinm[lo:hi].reshape(_BL * 128, _N)),
        }
        if has_bias:
            bq = np.asarray(b_qkv).astype(bf16)
            m["bqk"] = np.ascontiguousarray(bq[:2 * _C].reshape(1, 2 * _C))
            m["bv"] = np.ascontiguousarray(bq[2 * _C:].reshape(1, _C))
            m["bp"] = np.ascontiguousarray(
                np.asarray(b_proj).astype(bf16).reshape(1, _C)
            )
        in_maps.append(m)
    return in_maps, has_bias


def _ensure_ntff_hook():
    """This image's antenv lacks axon_hooks; recreate it from the boot
    helper so run_bass_kernel_spmd(trace=True) can capture NTFF profiles."""
    import sys
    import types

    if "antenv.axon_hooks" in sys.modules:
        return
    try:
        from trn_agent_boot.trn_boot import _ntff_profile_via_ctypes

        hook = _ntff_profile_via_ctypes("/opt/axon/libaxon_pjrt.so")
    except Exception:
        hook = None
    mod = types.ModuleType("antenv.axon_hooks")
    mod._hook = hook
    mod.get_axon_ntff_profile_hook = lambda: mod._hook
    mod.set_axon_ntff_profile_hook = lambda h: setattr(mod, "_hook", h)
    sys.modules["antenv.axon_hooks"] = mod


def run(x, W_qkv, b_qkv, W_proj, b_proj, pos_h, pos_w, num_heads, **run_kwargs):
    """Build + execute on 8 NeuronCores; returns (output, BassKernelResults)."""
    from concourse.bass_utils import run_bass_kernel_spmd

    if run_kwargs.get("trace"):
        _ensure_ntff_hook()

    assert int(num_heads) == _H
    in_maps, has_bias = _prep_in_maps(
        x, W_qkv, b_qkv, W_proj, b_proj, pos_h, pos_w
    )
    nc = _get_nc(has_bias)
    res = run_bass_kernel_spmd(
        nc, in_maps, core_ids=list(range(_NCORES)), **run_kwargs
    )
    out = np.concatenate(
        [np.asarray(res.results[i]["out"]).reshape(_BL, _N, _C)
         for i in range(_NCORES)],
        axis=0,
    ).astype(np.float32)
    return out, res


def kernel(x, W_qkv, b_qkv, W_proj, b_proj, pos_h, pos_w, num_heads):
    out, _ = run(x, W_qkv, b_qkv, W_proj, b_proj, pos_h, pos_w, num_heads)
    return out


# revision 40
# speedup vs baseline: 1.5439x; 1.2308x over previous
"""Trainium2 Bass kernel for nn_Attention_24704651887034.

Dense ViT-style attention block (B=16, N=1024, C=768, H=12 heads, 2D RoPE),
data-parallel over batch across 8 NeuronCores (2 batch items per core, no
collectives).

v2 — HAM-warm dense-PE schedule:
  * The PE array clock-gates to 1.2 GHz unless continuously busy (~3.4us
    windows).  The kernel therefore emits a single dense PE instruction
    stream: attention steps are interleaved with "filler" matmul closures
    (next item's QKV/V, previous item's proj) so the PE never idles while
    the Scalar engine computes softmax exps.
  * Score matmuls for the two heads of a pair are emitted adjacently on
    row-tiles (0,0)/(64,0) so the 64-contraction matmuls run concurrently
    in the two halves of the PE array.
  * exp is fused over kc-pairs ([128,1024] ACT ops) to halve ACT overhead.
  * Softmax denominators stay in PSUM row 64 ([v|1] ones trick); per
    (pair, q-half) they are inverted with the single-pass DVE
    reciprocal_approx_fast, DMA-broadcast, and fused into the PSUM->SBUF
    evacuation multiply (no separate normalize pass, nothing on the
    critical path at the proj boundary).
"""

from collections import deque

import numpy as np

_B, _N, _C, _H = 16, 1024, 768, 12
_HD, _DR = 64, 32
_ROPE_BASE = 10000.0
_NCORES = 8
_BL = _B // _NCORES  # batch items per core

_NC6 = _C // 128      # 6 contraction chunks

_nc_cache = {}


def _split_excess_waits(nc, max_waits=1):
    """Walrus in this toolchain accepts at most one sync-wait command per
    instruction; Tile's tail drain (and occasionally the scheduler) emits
    more. Split the excess onto same-engine NOPs inserted just before."""
    from concourse import mybir

    for f in nc.m.functions:
        for blk in f.blocks:
            insts = blk.instructions
            i = 0
            while i < len(insts):
                ins = insts[i]
                si = getattr(ins, "sync_info", None)
                if si is not None and len(si.on_wait) > max_waits:
                    excess = si.on_wait[max_waits:]
                    ins.sync_info = mybir.SyncInfo(
                        on_wait=list(si.on_wait[:max_waits]),
                        on_update=list(si.on_update),
                    )
                    for j, w in enumerate(excess):
                        nop = mybir.InstNoOp(
                            name=f"{ins.name}-sw{j}", engine=ins.engine
                        )
                        nop.sync_info = mybir.SyncInfo(on_wait=[w], on_update=[])
                        insts.insert(i, nop)
                        i += 1
                i += 1
    return nc


def _build(has_bias):
    from contextlib import ExitStack

    import concourse.bass as bass
    import concourse.tile as tile
    from concourse import mybir

    BF = mybir.dt.bfloat16
    F32 = mybir.dt.float32
    Exp = mybir.ActivationFunctionType.Exp
    N, C, H = _N, _C, _H
    BL = _BL

    nc = bass.Bass("TRN2", target_bir_lowering=False, debug=False)
    x_d = nc.dram_tensor("x", [BL * N, C], BF, kind="ExternalInput").ap()
    wq_d = nc.dram_tensor("wq", [C, 3 * C], BF, kind="ExternalInput").ap()
    wp_d = nc.dram_tensor("wp", [C, C], BF, kind="ExternalInput").ap()
    cos_d = nc.dram_tensor("cosm", [BL * 128, N], BF, kind="ExternalInput").ap()
    sin_d = nc.dram_tensor("ssinm", [BL * 128, N], BF, kind="ExternalInput").ap()
    if has_bias:
        bqk_d = nc.dram_tensor("bqk", [1, 2 * C], BF, kind="ExternalInput").ap()
        bv_d = nc.dram_tensor("bv", [1, C], BF, kind="ExternalInput").ap()
        bp_d = nc.dram_tensor("bp", [1, C], BF, kind="ExternalInput").ap()
    out_d = nc.dram_tensor("out", [BL * N, C], BF, kind="ExternalOutput").ap()

    SH_MASK = [(i + 16) % 32 for i in range(32)]
    F_ORDER = [0, 6, 1, 7, 2, 8, 3, 9, 4, 10, 5, 11]

    with tile.TileContext(nc) as tc, ExitStack() as ctx:
        const = ctx.enter_context(tc.tile_pool(name="const", bufs=1))
        xT_p = ctx.enter_context(tc.tile_pool(name="xT", bufs=2 * _NC6))
        map_p = ctx.enter_context(tc.tile_pool(name="maps", bufs=2))
        raw_p = ctx.enter_context(tc.tile_pool(name="qraw", bufs=3))
        tmp_p = ctx.enter_context(tc.tile_pool(name="rtmp", bufs=3))
        tmp2_p = ctx.enter_context(tc.tile_pool(name="rtmp2", bufs=3))
        qkr_p = ctx.enter_context(tc.tile_pool(name="qkr", bufs=22))
        v_p = ctx.enter_context(tc.tile_pool(name="v", bufs=16))
        pT_p = ctx.enter_context(tc.tile_pool(name="pT", bufs=4))
        ao_p = ctx.enter_context(tc.tile_pool(name="ao", bufs=12))
        s65_p = ctx.enter_context(tc.tile_pool(name="s65", bufs=8))
        rec_p = ctx.enter_context(tc.tile_pool(name="rec", bufs=3))
        rr2_p = ctx.enter_context(tc.tile_pool(name="rr2", bufs=3))
        bch_p = ctx.enter_context(tc.tile_pool(name="bch", bufs=3))
        ost_p = ctx.enter_context(tc.tile_pool(name="ost", bufs=3))
        mm_ps = ctx.enter_context(tc.tile_pool(name="mmps", bufs=2, space="PSUM"))
        sc_ps = ctx.enter_context(tc.tile_pool(name="scps", bufs=2, space="PSUM"))
        o_ps = ctx.enter_context(tc.tile_pool(name="ops", bufs=2, space="PSUM"))

        # ---- resident constants: weights ----
        wq_t = []
        for c in range(_NC6):
            t = const.tile([128, 3 * C], BF, tag=f"wq{c}")
            nc.sync.dma_start(t[:, 0:C], wq_d[c * 128:(c + 1) * 128, 0:C])
            wq_t.append(t)

        if has_bias:
            bqk_sb = const.tile([1, 2 * C], BF, tag="bqk")
            nc.sync.dma_start(bqk_sb[:], bqk_d[:])
            bv_sb = const.tile([1, C], BF, tag="bv")
            nc.sync.dma_start(bv_sb[:], bv_d[:])
            bp_sb = const.tile([1, C], BF, tag="bp")
            nc.sync.dma_start(bp_sb[:], bp_d[:])
            ones_r = const.tile([1, 512], BF, tag="ones")
            nc.gpsimd.memset(ones_r[:], 1.0)

        # ---- per-item input DMAs, ordered by first use ----
        xT = {}
        cosm = {}
        ssin = {}

        def emit_item_inputs(b):
            for c in range(_NC6):
                t = xT_p.tile([128, N], BF, tag="xT", name=f"xT{b}_{c}")
                nc.sync.dma_start(
                    t[:], x_d[b * N:(b + 1) * N, c * 128:(c + 1) * 128],
                    transpose=True,
                )
                xT[b, c] = t
            m = map_p.tile([128, N], BF, tag="cos", name=f"cos{b}")
            nc.sync.dma_start(m[:], cos_d[b * 128:(b + 1) * 128, :])
            cosm[b] = m
            m = map_p.tile([128, N], BF, tag="sin", name=f"sin{b}")
            nc.sync.dma_start(m[:], sin_d[b * 128:(b + 1) * 128, :])
            ssin[b] = m

        emit_item_inputs(0)
        for c in range(_NC6):
            nc.sync.dma_start(
                wq_t[c][:, C:2 * C], wq_d[c * 128:(c + 1) * 128, C:2 * C]
            )
        for c in range(_NC6):
            nc.sync.dma_start(
                wq_t[c][:, 2 * C:3 * C], wq_d[c * 128:(c + 1) * 128, 2 * C:3 * C]
            )
        wp_t = []
        for c in range(_NC6):
            t = const.tile([128, C], BF, tag=f"wp{c}")
            nc.sync.dma_start(t[:], wp_d[c * 128:(c + 1) * 128, :])
            wp_t.append(t)
        emit_item_inputs(1)

        qk_r = {}
        v_sb = {}
        ao = {}

        # ---- emitters ----
        def emit_q_half(b, f, t2, raw):
            ps = mm_ps.tile([128, 512], F32, tag="mm", name=f"q{b}_{f}_{t2}")
            for c in range(_NC6):
                nc.tensor.matmul(
                    ps[:],
                    wq_t[c][:, f * 128:(f + 1) * 128],
                    xT[b, c][:, t2 * 512:(t2 + 1) * 512],
                    start=(c == 0),
                    stop=(c == _NC6 - 1 and not has_bias),
                )
            if has_bias:
                nc.tensor.matmul(
                    ps[:],
                    bqk_sb[:, f * 128:(f + 1) * 128],
                    ones_r[:],
                    start=False,
                    stop=True,
                )
            nc.scalar.copy(raw[:, t2 * 512:(t2 + 1) * 512], ps[:])

        def emit_q_rope(b, f, raw, inline=False):
            st2 = {}
            ro = qkr_p.tile([128, N], BF, tag="qkr", name=f"qkr{b}_{f}")
            qk_r[b, f] = ro

            def rope():
                # all on DVE: no cross-engine waits inside the chain, so it
                # never blocks the FIFO once its raw input is resident
                r = tmp_p.tile([128, N], BF, tag="rt", name=f"rr{b}_{f}")
                nc.vector.stream_shuffle(r[:], raw[:], SH_MASK)
                tm = tmp2_p.tile([128, N], BF, tag="rm", name=f"rm{b}_{f}")
                nc.vector.tensor_mul(tm[:], r[:], ssin[b][:])
                am = tmp_p.tile([128, N], BF, tag="rt", name=f"ra{b}_{f}")
                nc.vector.tensor_mul(am[:], raw[:], cosm[b][:])
                nc.vector.tensor_add(ro[:], am[:], tm[:])
                st2["done"] = True

            if inline:
                rope()
            else:
                defer(2, lambda: ("done" in st2) or rope(),
                      key=("qadd", b, f))

        def mk_q_closures(b, f):
            st = {}

            def c1():
                st["raw"] = raw_p.tile([128, N], BF, tag="qraw", name=f"qw{b}_{f}")
                emit_q_half(b, f, 0, st["raw"])

            def c2():
                emit_q_half(b, f, 1, st["raw"])
                emit_q_rope(b, f, st["raw"])

            return [(("q", b, f), c1), (("q", b, f), c2)]

        def mk_v_closures(b, t8):
            st = {}

            def grp(f0, fw, first):
                vt = st["vt"]
                vt3 = vt.rearrange("p (h w) -> p h w", w=65)
                if first:
                    nc.gpsimd.memset(vt3[:, :, 64:65], 1.0)
                ps = mm_ps.tile([128, 512], F32, tag="mm", name=f"v{b}_{t8}_{f0}")
                for c in range(_NC6):
                    nc.tensor.matmul(
                        ps[:, :fw],
                        xT[b, c][:, t8 * 128:(t8 + 1) * 128],
                        wq_t[c][:, 2 * C + f0:2 * C + f0 + fw],
                        start=(c == 0),
                        stop=(c == _NC6 - 1 and not has_bias),
                    )
                if has_bias:
                    nc.tensor.matmul(
                        ps[:, :fw],
                        ones_r[:, t8 * 128 % 512:t8 * 128 % 512 + 128],
                        bv_sb[:, f0:f0 + fw],
                        start=False,
                        stop=True,
                    )
                nh = fw // 64
                nc.vector.tensor_copy(
                    vt3[:, f0 // 64:f0 // 64 + nh, 0:64],
                    ps[:, :fw].rearrange("p (h w) -> p h w", w=64),
                )

            def c1():
                st["vt"] = v_p.tile([128, H * 65], BF, tag="v", name=f"v{b}_{t8}")
                v_sb[b, t8] = st["vt"]
                grp(0, 512, True)

            def c2():
                grp(512, 256, False)

            return [(("v", b, t8), c1), (("v", b, t8), c2)]

        def mk_p_closure(b, t8, nf):
            def c1():
                ps = mm_ps.tile([128, 512], F32, tag="mm", name=f"p{b}_{t8}_{nf}")
                for jj in range(_NC6):
                    nc.tensor.matmul(
                        ps[:, :384],
                        ao[b, jj][:, t8 * 128:(t8 + 1) * 128],
                        wp_t[jj][:, nf * 384:(nf + 1) * 384],
                        start=(jj == 0),
                        stop=(jj == _NC6 - 1 and not has_bias),
                    )
                if has_bias:
                    nc.tensor.matmul(
                        ps[:, :384],
                        ones_r[:, 0:128],
                        bp_sb[:, nf * 384:(nf + 1) * 384],
                        start=False,
                        stop=True,
                    )
                ot = ost_p.tile([128, 384], BF, tag="ost", name=f"ot{b}_{t8}_{nf}")
                nc.scalar.copy(ot[:], ps[:, :384])

                def odma():
                    nc.sync.dma_start(
                        out_d[b * N + t8 * 128:b * N + (t8 + 1) * 128,
                              nf * 384:(nf + 1) * 384],
                        ot[:],
                    )

                defer(2, odma)

            return [(("p", b, t8, nf), c1)]

        # ---- filler + deferred-emission machinery ----
        # fillers: (key, closure) of PE work pulled between attention steps.
        # deferred: (due_point, closure) of non-PE tail work (rope adds,
        # reciprocal chains) emitted a few fill-points late so their deps
        # are ready when they enter their engine FIFO (no head-of-line
        # blocking).
        fillers = deque()
        deferred = []
        fstate = {"acc": 0.0, "ratio": 0.0, "pt": 0}

        def set_fill_ratio(n_points):
            fstate["ratio"] = (1.4 * len(fillers) / n_points) if n_points else 0.0
            fstate["acc"] = 0.0

        def defer(delay, fn, key=None):
            deferred.append((fstate["pt"] + delay, fn, key))

        def force_defer(keys):
            for item in [x for x in deferred]:
                if item[2] in keys:
                    deferred.remove(item)
                    item[1]()

        def fill(drain=True):
            fstate["pt"] += 1
            if drain:
                for item in [x for x in deferred]:
                    if item[0] <= fstate["pt"]:
                        deferred.remove(item)
                        item[1]()
            fstate["acc"] += fstate["ratio"]
            while fstate["acc"] >= 1.0 and fillers:
                fillers.popleft()[1]()
                fstate["acc"] -= 1.0

        def force_fill(keys):
            for item in [x for x in fillers]:
                if item[0] in keys:
                    fillers.remove(item)
                    item[1]()

        def flush_fillers():
            while fillers:
                fillers.popleft()[1]()
            # deferred closures can enqueue more (staged chains): drain fully
            while deferred:
                deferred.sort(key=lambda x: x[0])
                item = deferred.pop(0)
                item[1]()

        # ---- attention: flat software-pipelined stream ----
        # The scores/exp cursor leads the AV cursor by 2 kc-steps ACROSS
        # block boundaries, so the first AVs of a block land ~2 PE steps
        # after the previous block's PSUM evacuations (no o-bank WAR stall).
        def emit_att_item(b, pairs):
            state = {}

            def s_unit(u):
                j, qh, kc = u
                if kc == 0 and qh == 0:
                    force_fill({("q", b, j), ("q", b, 6 + j)})
                    force_defer({("qadd", b, j), ("qadd", b, 6 + j)})
                    if (b, j) not in ao:
                        ao[b, j] = ao_p.tile([128, N], BF, tag="ao",
                                             name=f"ao{b}_{j}")
                qT = qk_r[b, j]
                kT = qk_r[b, 6 + j]
                qsl = slice(qh * 512, (qh + 1) * 512)
                sc = sc_ps.tile([128, 1024], F32, tag="sc",
                                name=f"s{b}_{j}_{qh}_{kc}")
                for h2 in range(2):
                    half = h2 * 64
                    nc.tensor.matmul(
                        sc[:, h2 * 512:(h2 + 1) * 512],
                        kT[half:half + 64, kc * 128:(kc + 1) * 128],
                        qT[half:half + 64, qsl],
                        start=True,
                        stop=True,
                    )
                pt = pT_p.tile([128, 1024], BF, tag="pt",
                               name=f"pt{b}_{j}_{qh}_{kc}")
                nc.scalar.activation(pt[:], sc[:], Exp, scale=0.125)
                state[(j, qh, kc)] = pt

            def a_unit(u):
                j, qh, kd = u
                if kd == 0:
                    state[(j, qh, "o")] = [
                        o_ps.tile([65, 512], F32, tag="o",
                                  name=f"o{b}_{j}_{qh}_{h2}")
                        for h2 in range(2)
                    ]
                o_t = state[(j, qh, "o")]
                pt = state.pop((j, qh, kd))
                for h2 in range(2):
                    h = 2 * j + h2
                    vt3 = v_sb[b, kd].rearrange("p (h w) -> p h w", w=65)
                    nc.tensor.matmul(
                        o_t[h2][:],
                        vt3[:, h, 0:65],
                        pt[:, h2 * 512:(h2 + 1) * 512],
                        start=(kd == 0),
                        stop=(kd == 7),
                    )
                if kd == 7:
                    emit_den(b, j, qh, o_t)

            S = [(j, qh, kc) for j in pairs for qh in range(2)
                 for kc in range(8)]
            A = [(j, qh, kd) for j in pairs for qh in range(2)
                 for kd in range(8)]
            LOOK = 2
            for t in range(len(A) + LOOK):
                if t < len(S):
                    s_unit(S[t])
                if t >= LOOK:
                    a_unit(A[t - LOOK])
                if t % 2 == 0:
                    # no deferred-drain right before a block's last AVs:
                    # the PSUM evacuations must enter the DVE FIFO first
                    fill(drain=((t - LOOK) % 8) != 6)
        # evacuate [v-rows | denominator] to SBUF, freeing the PSUM
        # banks with single DVE copies; the reciprocal/broadcast/
        # normalize chain is emitted in deferred stages so that by
        # the time each op enters its engine FIFO its inputs are
        # already resident (no cross-engine head-of-line blocking).
        def emit_den(b, j, qh, o_t):
            qsl = slice(qh * 512, (qh + 1) * 512)
            s65s = []
            for h2 in range(2):
                nm = f"{b}_{j}_{qh}_{h2}"
                s65 = s65_p.tile([65, 512], BF, tag="s65", name=f"e{nm}")
                nc.vector.tensor_copy(s65[:], o_t[h2][:])
                s65s.append(s65)
            d32 = rec_p.tile([32, 32], BF, tag="d32",
                             name=f"d{b}_{j}_{qh}")
            for h2 in range(2):
                nc.gpsimd.dma_start(
                    d32[h2 * 16:(h2 + 1) * 16, :],
                    s65s[h2][64:65, :].rearrange("p (a w) -> p a w", a=16),
                )

            def mk_stage2():
                d32r = rec_p.tile([32, 32], F32, tag="d32r",
                                  name=f"r{b}_{j}_{qh}")
                nc.vector.reciprocal(d32r[:], d32[:])
                rr2 = rr2_p.tile([2, 512], F32, tag="rr2",
                                 name=f"w{b}_{j}_{qh}")
                nc.sync.dma_start(
                    rr2[:].rearrange("p (a w) -> p a w", a=16), d32r[:]
                )

                def mk_stage3():
                    bchs = []
                    for h2 in range(2):
                        bch = bch_p.tile([64, 512], F32, tag="bch",
                                         name=f"bc{b}_{j}_{qh}_{h2}")
                        nc.sync.dma_start(
                            bch[:],
                            rr2[h2:h2 + 1, :]
                            .rearrange("p (u n) -> p u n", u=1)
                            .broadcast_to((1, 64, 512)),
                        )
                        bchs.append(bch)

                    def mk_stage4():
                        for h2 in range(2):
                            nc.gpsimd.tensor_mul(
                                ao[b, j][h2 * 64:h2 * 64 + 64, qsl],
                                s65s[h2][0:64, :], bchs[h2][:],
                            )

                    defer(7, mk_stage4)

                defer(7, mk_stage3)

            defer(14, mk_stage2)

        # ================= main schedule =================
        # item 0: only pair-0 QKV chunks and V inline; the remaining ten
        # QKV chunks become fillers so attention (and the ACT exp stream)
        # starts ~20us earlier.
        for f in [0, 6]:
            raw = raw_p.tile([128, N], BF, tag="qraw", name=f"qw0_{f}")
            emit_q_half(0, f, 0, raw)
            emit_q_half(0, f, 1, raw)
            emit_q_rope(0, f, raw, inline=True)
        for t8 in range(8):
            for _, c in mk_v_closures(0, t8):
                c()

        # attention item 0: filled with item-0 late QKV, item-1 V + early QKV
        for f in [1, 7, 2, 8, 3, 9, 4, 10, 5, 11]:
            fillers.extend(mk_q_closures(0, f))
        for f in [0, 6, 1, 7, 2, 8]:
            fillers.extend(mk_q_closures(1, f))
        for t8 in range(8):
            fillers.extend(mk_v_closures(1, t8))
        set_fill_ratio(7 * 12)
        emit_att_item(0, list(range(_NC6)))
        while fillers:
            fillers.popleft()[1]()

        # attention item 1: filled with item-1 late QKV chunks + item-0 proj
        for f in [3, 9, 4, 10, 5, 11]:
            fillers.extend(mk_q_closures(1, f))
        for t8 in range(8):
            for nf in range(2):
                fillers.extend(mk_p_closure(0, t8, nf))
        set_fill_ratio(7 * 12)
        emit_att_item(1, list(range(_NC6)))
        flush_fillers()

        # proj item 1 inline
        for t8 in range(8):
            for nf in range(2):
                for _, c in mk_p_closure(1, t8, nf):
                    c()
        flush_fillers()
    return _split_excess_waits(nc)


def _get_nc(has_bias):
    if has_bias not in _nc_cache:
        _nc_cache[has_bias] = _build(has_bias)
    return _nc_cache[has_bias]


def _prep_in_maps(x, W_qkv, b_qkv, W_proj, b_proj, pos_h, pos_w):
    import ml_dtypes

    bf16 = ml_dtypes.bfloat16
    has_bias = bool(np.any(b_qkv)) or bool(np.any(b_proj))

    inv = 1.0 / _ROPE_BASE ** (
        np.arange(0, _DR, 2, dtype=np.float32) / float(_DR)
    )  # [16]

    def rope_maps(pos):
        ang = pos.astype(np.float32)[..., None] * inv  # [B, N, 16]
        cos = np.repeat(np.cos(ang), 2, axis=-1)  # [B, N, 32]
        sin = np.repeat(np.sin(ang), 2, axis=-1)
        return cos.transpose(0, 2, 1), sin.transpose(0, 2, 1)  # [B, 32, N]

    ch, sh = rope_maps(np.asarray(pos_h))
    cw, sw = rope_maps(np.asarray(pos_w))
    cos64 = np.concatenate([ch, cw], axis=1)  # [B, 64, N]
    sin64 = np.concatenate([sh, sw], axis=1)
    sign = np.where((np.arange(64) % 32) < 16, -1.0, 1.0).astype(np.float32)
    ssin64 = sin64 * sign[None, :, None]
    cosm = np.tile(cos64, (1, 2, 1)).astype(bf16)  # [B, 128, N]
    ssinm = np.tile(ssin64, (1, 2, 1)).astype(bf16)

    xb = np.asarray(x).astype(bf16)
    wqb = np.ascontiguousarray(np.asarray(W_qkv).astype(bf16))
    wpb = np.ascontiguousarray(np.asarray(W_proj).astype(bf16))

    in_maps = []
    for i in range(_NCORES):
        lo, hi = i * _BL, (i + 1) * _BL
        m = {
            "x": np.ascontiguousarray(xb[lo:hi].reshape(_BL * _N, _C)),
            "wq": wqb,
            "wp": wpb,
            "cosm": np.ascontiguousarray(cosm[lo:hi].reshape(_BL * 128, _N)),
            "ssinm": np.ascontiguousarray(ssinm[lo:hi].reshape(_BL * 128, _N)),
        }
        if has_bias:
            bq = np.asarray(b_qkv).astype(bf16)
            m["bqk"] = np.ascontiguousarray(bq[:2 * _C].reshape(1, 2 * _C))
            m["bv"] = np.ascontiguousarray(bq[2 * _C:].reshape(1, _C))
            m["bp"] = np.ascontiguousarray(
                np.asarray(b_proj).astype(bf16).reshape(1, _C)
            )
        in_maps.append(m)
    return in_maps, has_bias


def _ensure_ntff_hook():
    """This image's antenv lacks axon_hooks; recreate it from the boot
    helper so run_bass_kernel_spmd(trace=True) can capture NTFF profiles."""
    import sys
    import types

    if "antenv.axon_hooks" in sys.modules:
        return
    try:
        from trn_agent_boot.trn_boot import _ntff_profile_via_ctypes

        hook = _ntff_profile_via_ctypes("/opt/axon/libaxon_pjrt.so")
    except Exception:
        hook = None
    mod = types.ModuleType("antenv.axon_hooks")
    mod._hook = hook
    mod.get_axon_ntff_profile_hook = lambda: mod._hook
    mod.set_axon_ntff_profile_hook = lambda h: setattr(mod, "_hook", h)
    sys.modules["antenv.axon_hooks"] = mod


def run(x, W_qkv, b_qkv, W_proj, b_proj, pos_h, pos_w, num_heads, **run_kwargs):
    """Build + execute on 8 NeuronCores; returns (output, BassKernelResults)."""
    from concourse.bass_utils import run_bass_kernel_spmd

    if run_kwargs.get("trace"):
        _ensure_ntff_hook()

    assert int(num_heads) == _H
    in_maps, has_bias = _prep_in_maps(
        x, W_qkv, b_qkv, W_proj, b_proj, pos_h, pos_w
    )
    nc = _get_nc(has_bias)
    res = run_bass_kernel_spmd(
        nc, in_maps, core_ids=list(range(_NCORES)), **run_kwargs
    )
    out = np.concatenate(
        [np.asarray(res.results[i]["out"]).reshape(_BL, _N, _C)
         for i in range(_NCORES)],
        axis=0,
    ).astype(np.float32)
    return out, res


def kernel(x, W_qkv, b_qkv, W_proj, b_proj, pos_h, pos_w, num_heads):
    out, _ = run(x, W_qkv, b_qkv, W_proj, b_proj, pos_h, pos_w, num_heads)
    return out
